# revision 1
# baseline (speedup 1.0000x reference)
"""Linear-attention MultiHeadAttentionBlock kernel for 8 Trainium2 NeuronCores.

Sharding: core c handles (batch b = c//2, head-group g = c%2).  Each core
computes, for its batch's q/k/v and its 8 heads (512 of the 1024 d_model
dims):
    QfT = elu(Wq_g @ X_q^T)+1          (transposed space: d' on partitions)
    Kf  = elu(X_k @ Wk_g^T)+1          (n-space)
    Vp  = X_v @ Wv_g^T                 (n-space)
    per head-pair dt: KV block + ksum (PE accumulation over n)
    Zpre = Qf . ksum  -> zr = 1/Zpre   (eps is negligible: Zpre ~ 1e5 > 0)
    out  = (Qf @ KV) * zr              (n-space), transposed on PE,
    y    = out_g @ WoS^T               (partial d_model-1024 output)
Host sums the two per-batch partials.

Matmuls run in float32r (TF32-like fast fp32 mode on the PE; set
MM_DTYPE=float32 env to force exact fp32 at 4x the PE cost).
"""

import os

import numpy as np

import concourse.bass as bass
import concourse.mybir as mybir
import concourse.tile as tile
from concourse import bacc
from concourse.bass_utils import run_bass_kernel_spmd
from concourse.masks import make_identity

P = 128
L = 2048          # sequence length
DM = 1024         # d_model (= contraction dim of projections)
DG = 512          # per-core head-group width (8 heads x 64)
NT = L // P       # 16 n-tiles
KC = DM // P      # 8 contraction chunks
DT = DG // P      # 4 d'-tiles (2 heads each)
NCH = 4           # n-chunks of 512 for transposed-Q projection
F32 = mybir.dt.float32

_CACHE = {}


def _mm_dtype():
    name = os.environ.get("MM_DTYPE", "float32r")
    return {"float32r": mybir.dt.float32r, "float32": F32}[name]


def build_nc(repeats=1):
    MMDT = _mm_dtype()
    nc = bacc.Bacc(None, target_bir_lowering=False)

    xq_d = nc.dram_tensor("xqT", [DM, L], MMDT, kind="ExternalInput")
    xk_d = nc.dram_tensor("xkT", [DM, L], MMDT, kind="ExternalInput")
    xv_d = nc.dram_tensor("xvT", [DM, L], MMDT, kind="ExternalInput")
    wq_d = nc.dram_tensor("wqT", [DM, DG], MMDT, kind="ExternalInput")
    wk_d = nc.dram_tensor("wkT", [DM, DG], MMDT, kind="ExternalInput")
    wv_d = nc.dram_tensor("wvT", [DM, DG], MMDT, kind="ExternalInput")
    wo_d = nc.dram_tensor("woT", [DG, DM], MMDT, kind="ExternalInput")
    sel_d = nc.dram_tensor("sel8", [32, P], MMDT, kind="ExternalInput")
    y_d = nc.dram_tensor("y", [DM, L], F32, kind="ExternalOutput")

    Exp = mybir.ActivationFunctionType.Exp
    Relu = mybir.ActivationFunctionType.Relu
    Alu = mybir.AluOpType

    with tile.TileContext(nc) as tc:
        with (
            tc.tile_pool(name="const", bufs=1) as cpool,
            tc.tile_pool(name="xt", bufs=17) as xt,      # (128,1024) input half-chunks
            tc.tile_pool(name="wt", bufs=8) as wt,       # (128,512) weight chunks
            tc.tile_pool(name="wo", bufs=4) as wop,      # (128,1024) w_o chunks
            tc.tile_pool(name="qft", bufs=16) as qftp,   # QfT persistent
            tc.tile_pool(name="kf", bufs=16) as kfp,     # Kf, later out_sb
            tc.tile_pool(name="vp", bufs=3) as vpp,      # Vp rotating
            tc.tile_pool(name="tmp", bufs=3) as tmp,     # feature-map temps
            tc.tile_pool(name="kvsb", bufs=1) as kvsb,   # KVcat2/KSUM2/zrb
            tc.tile_pool(name="ysb", bufs=2) as ysb,     # (128,2048) y row-batches
            tc.tile_pool(name="pp", bufs=4, space="PSUM") as pp,    # (128,512)
            tc.tile_pool(name="kvp", bufs=4, space="PSUM") as kvp,  # bank-sized
        ):
            ident = cpool.tile([P, P], F32, name="ident")
            make_identity(nc, ident[:])
            ones = cpool.tile([P, 2], MMDT, name="ones")
            ones_f = cpool.tile([P, 2], F32, name="ones_f")
            nc.gpsimd.memset(ones_f[:], 1.0)
            nc.vector.tensor_copy(ones[:], ones_f[:])
            # sel8[dt]: (8,128) mask, row 2dt+s has 1s in columns [64s,64s+64)
            sel8 = []
            for _dt in range(4):
                st = cpool.tile([8, P], MMDT, name=f"sel8_{_dt}")
                nc.sync.dma_start(st[:], sel_d[_dt * 8:(_dt + 1) * 8, :])
                sel8.append(st)
            for _rep in range(repeats):
                body(nc, tc, MMDT, ident, ones, sel8,
                     xt, wt, wop, qftp, kfp, vpp, tmp, kvsb, ysb, pp, kvp,
                     xq_d, xk_d, xv_d, wq_d, wk_d, wv_d, wo_d, y_d)

    nc.compile()
    return nc


def body(nc, tc, MMDT, ident, ones, sel8,
         xt, wt, wop, qftp, kfp, vpp, tmp, kvsb, ysb, pp, kvp,
         xq_d, xk_d, xv_d, wq_d, wk_d, wv_d, wo_d, y_d):
    Exp = mybir.ActivationFunctionType.Exp
    Relu = mybir.ActivationFunctionType.Relu
    Alu = mybir.AluOpType
    F32_ = F32
    if True:
        if True:
            def feature_map(ps, dst):
                # dst = elu(ps)+1 = exp(min(ps,0)) + relu(ps)
                t0 = tmp.tile([P, 512], F32, tag="tmp", name="t0")
                t1 = tmp.tile([P, 512], F32, tag="tmp", name="t1")
                nc.vector.tensor_scalar(t0[:], ps[:], 0.0, None, Alu.min)
                nc.scalar.activation(t1[:], ps[:], Relu)
                nc.scalar.activation(dst[:], t0[:], Exp)
                nc.vector.tensor_tensor(dst[:], dst[:], t1[:], Alu.add)

            # ---------------- Phase Q: QfT (transposed) ----------------
            # x tensors stream as half-chunks (128, 1024) so each half-phase
            # frees its slots early and the next phase's DMA runs ahead.
            def dma_x_half(src_d, half, kc):
                t = xt.tile([P, L // 2], MMDT, tag="xt", name="xh")
                nc.sync.dma_start(
                    t[:], src_d[kc * P:(kc + 1) * P,
                                half * (L // 2):(half + 1) * (L // 2)])
                return t

            wq = []
            xqh = [[None] * KC, [None] * KC]
            for kc in range(KC):
                xqh[0][kc] = dma_x_half(xq_d, 0, kc)
                wt_t = wt.tile([P, DG], MMDT, tag="wt")
                nc.sync.dma_start(wt_t[:], wq_d[kc * P:(kc + 1) * P, :])
                wq.append(wt_t)
            for kc in range(KC):
                xqh[1][kc] = dma_x_half(xq_d, 1, kc)
            qft = [None] * 16  # (128, 512) tiles: index dt*4+nch
            for half in range(2):
                for dt in range(DT):
                    ps0 = pp.tile([P, 512], F32, tag="pp", name="ps0")
                    ps1 = pp.tile([P, 512], F32, tag="pp", name="ps1")
                    for kc in range(KC):
                        # same stationary wq block for both n-chunks
                        nc.tensor.matmul(
                            ps0[:],
                            wq[kc][:, dt * P:(dt + 1) * P],
                            xqh[half][kc][:, 0:512],
                            start=(kc == 0), stop=(kc == KC - 1),
                        )
                        nc.tensor.matmul(
                            ps1[:],
                            wq[kc][:, dt * P:(dt + 1) * P],
                            xqh[half][kc][:, 512:1024],
                            start=(kc == 0), stop=(kc == KC - 1),
                        )
                    for sub, ps in ((0, ps0), (1, ps1)):
                        qf = qftp.tile([P, 512], MMDT, tag="qft")
                        feature_map(ps, qf)
                        qft[dt * NCH + half * 2 + sub] = qf

            def qft_block(dt, nt):
                # lhsT (128, 128): QfT[dt-block][:, nt*128 : nt*128+128]
                t = qft[dt * NCH + (nt * P) // 512]
                off = (nt * P) % 512
                return t[:, off:off + P]

            # ---------------- Phase K: Kf (n-space) ----------------
            wk = []
            xkh = [[None] * KC, [None] * KC]
            for kc in range(KC):
                xkh[0][kc] = dma_x_half(xk_d, 0, kc)
                wt_t = wt.tile([P, DG], MMDT, tag="wt")
                nc.sync.dma_start(wt_t[:], wk_d[kc * P:(kc + 1) * P, :])
                wk.append(wt_t)
            for kc in range(KC):
                xkh[1][kc] = dma_x_half(xk_d, 1, kc)
            kf = []
            ksumb = [kvp.tile([P, 2], F32, tag="acc", name=f"ksumb{_d}")
                     for _d in range(DT)]
            for nt in range(NT):
                half, sub = nt // 8, nt % 8
                ps = pp.tile([P, 512], F32, tag="pp")
                for kc in range(KC):
                    nc.tensor.matmul(
                        ps[:],
                        xkh[half][kc][:, sub * P:(sub + 1) * P],
                        wk[kc][:],
                        start=(kc == 0), stop=(kc == KC - 1),
                    )
                kft = kfp.tile([P, 512], MMDT, tag="kf")
                feature_map(ps, kft)
                kf.append(kft)
                for dt in range(DT):
                    nc.tensor.matmul(
                        ksumb[dt][:],
                        kft[:, dt * P:(dt + 1) * P],
                        ones[:],
                        start=(nt == 0), stop=(nt == NT - 1),
                    )

            # ksum2 columns from ksumb; then Z-prep (zbank/recip/zrA), which
            # overlaps the V-phase input DMA wait
            ksum2 = kvsb.tile([P, 2 * DT], MMDT, tag="ksum2")
            for dt in range(DT):
                nc.scalar.copy(ksum2[0:64, 2 * dt:2 * dt + 1],
                               ksumb[dt][0:64, 0:1])
                nc.scalar.mul(ksum2[64:128, 2 * dt:2 * dt + 1],
                              ksumb[dt][64:128, 0:1], 0.0)
                nc.scalar.mul(ksum2[0:64, 2 * dt + 1:2 * dt + 2],
                              ksumb[dt][0:64, 0:1], 0.0)
                nc.scalar.copy(ksum2[64:128, 2 * dt + 1:2 * dt + 2],
                               ksumb[dt][64:128, 0:1])

            zbank = kvp.tile([P, P], F32, tag="acc", name="zbank")
            idx = 0
            for nt in range(NT):
                for dt in range(DT):
                    c = nt * 8 + dt * 2
                    nc.tensor.matmul(
                        zbank[:, c:c + 2],
                        qft_block(dt, nt),
                        ksum2[:, 2 * dt:2 * dt + 2],
                        start=(idx == 0), stop=(idx == NT * DT - 1),
                        skip_group_check=True,
                    )
                    idx += 1
            zrb = kvsb.tile([P, P], F32, tag="zrb", name="zrb")
            nc.vector.reciprocal(zrb[:], zbank[:])
            zrA = kvsb.tile([8, L], MMDT, tag="zrA", name="zrA")
            for nt in range(NT):
                ztp = kvp.tile([8, P], F32, tag="acc", name="ztp")
                nc.tensor.transpose(ztp[:], zrb[:, nt * 8:(nt + 1) * 8], ident[:])
                nc.vector.tensor_copy(zrA[:, nt * P:(nt + 1) * P], ztp[:])

            # ---------------- Phase V + KV accumulation ----------------
            wv = []
            xvh = [[None] * KC, [None] * KC]
            for kc in range(KC):
                xvh[0][kc] = dma_x_half(xv_d, 0, kc)
                wt_t = wt.tile([P, DG], MMDT, tag="wt")
                nc.sync.dma_start(wt_t[:], wv_d[kc * P:(kc + 1) * P, :])
                wv.append(wt_t)
            for kc in range(KC):
                xvh[1][kc] = dma_x_half(xv_d, 1, kc)
            # KVS2[dt] (128, 256) = Kf_dt^T @ Vp_(256-pair): diag block is KV
            kvs = [kvp.tile([P, 2 * P], F32, tag="acc", name=f"kvs{_dt}") for _dt in range(DT)]
            for nt in range(NT):
                half, sub = nt // 8, nt % 8
                ps = pp.tile([P, 512], F32, tag="pp")
                for kc in range(KC):
                    nc.tensor.matmul(
                        ps[:],
                        xvh[half][kc][:, sub * P:(sub + 1) * P],
                        wv[kc][:],
                        start=(kc == 0), stop=(kc == KC - 1),
                    )
                vp_t = vpp.tile([P, 512], MMDT, tag="vp")
                nc.scalar.copy(vp_t[:], ps[:])
                for dt in range(DT):
                    nc.tensor.matmul(
                        kvs[dt][:],
                        kf[nt][:, dt * P:(dt + 1) * P],
                        vp_t[:, (dt // 2) * 2 * P:(dt // 2 + 1) * 2 * P],
                        start=(nt == 0), stop=(nt == NT - 1),
                    )
                # interleave QfT Z-scaling (exact: KVcat2 is block-diagonal,
                # so scaling Qf by zr[n, head] == scaling out by zr)
                if nt % 4 == 3:
                    nch = nt // 4
                    for dt in range(DT):
                        zrp = pp.tile([P, 512], F32, tag="pp", name="zrp")
                        nc.tensor.matmul(
                            zrp[:], sel8[dt][:],
                            zrA[:, nch * 512:(nch + 1) * 512],
                            start=True, stop=True,
                        )
                        zrs = tmp.tile([P, 512], F32, tag="tmp", name="zrs")
                        nc.scalar.copy(zrs[:], zrp[:])
                        qt = qft[dt * NCH + nch]
                        nc.vector.tensor_tensor(qt[:], qt[:], zrs[:],
                                                mybir.AluOpType.mult)

            # KVcat2[dt]: block-diag [KV_A, KV_B] from the 256-wide KVS2
            kvcat = []
            for dt in range(DT):
                off = (dt % 2) * P
                kvc = kvsb.tile([P, P], MMDT, tag="kvcat", bufs=4)
                nc.scalar.copy(kvc[0:64, 0:64],
                               kvs[dt][0:64, off:off + 64])
                nc.scalar.mul(kvc[0:64, 64:128],
                              kvs[dt][0:64, off + 64:off + 128], 0.0)
                nc.scalar.mul(kvc[64:128, 0:64],
                              kvs[dt][64:128, off:off + 64], 0.0)
                nc.scalar.copy(kvc[64:128, 64:128],
                               kvs[dt][64:128, off + 64:off + 128])
                kvcat.append(kvc)

            # wo chunks (needed soon)
            wo = []
            for dc in range(DT):
                wo_t = wop.tile([P, DM], MMDT, tag="wo", name="wo_t")
                nc.sync.dma_start(wo_t[:], wo_d[dc * P:(dc + 1) * P, :])
                wo.append(wo_t)

            # ---------------- transposed out + Z-scale + final ----------------
            # outT[(dt, nch)]: (128 m'-part, 512 n) = (KVcat2[dt]^T @ QfT) * zrep
            def out_chain(nch):
                outT_n = []
                for dt in range(DT):
                    otp = pp.tile([P, 512], F32, tag="pp", name="otp")
                    nc.tensor.matmul(
                        otp[:], kvcat[dt][:],
                        qft[dt * NCH + nch][:],
                        start=True, stop=True,
                    )
                    oT = kfp.tile([P, 512], MMDT, tag="kf", name="oT")
                    if (nch + dt) % 2 == 0:
                        nc.vector.tensor_copy(oT[:], otp[:])
                    else:
                        nc.scalar.copy(oT[:], otp[:])
                    outT_n.append(oT)
                return outT_n

            outT_all = []
            for nch in range(NCH):
                outT_all.append(out_chain(nch))

            # final projection, yT = WoS @ out_g^T: wo blocks stationary,
            # each reused across the 4 n-chunks
            for jb in range(8):
                ypool, ytag = ((kvp, "acc") if jb % 2 == 0 else (pp, "pp"))
                yps = [ypool.tile([P, 512], F32, tag=ytag, name=f"yp{_n}")
                       for _n in range(NCH)]
                for dc in range(DT):
                    for nch in range(NCH):
                        nc.tensor.matmul(
                            yps[nch][:],
                            wo[dc][:, jb * P:(jb + 1) * P],
                            outT_all[nch][dc][:],
                            start=(dc == 0), stop=(dc == DT - 1),
                        )
                yt = ysb.tile([P, L], F32, tag="ysb", name="yt")
                for nch in range(NCH):
                    if (jb + nch) % 2 == 0:
                        nc.vector.tensor_copy(
                            yt[:, nch * 512:(nch + 1) * 512], yps[nch][:])
                    else:
                        nc.scalar.copy(
                            yt[:, nch * 512:(nch + 1) * 512], yps[nch][:])
                nc.sync.dma_start(y_d[jb * P:(jb + 1) * P, :], yt[:])


def make_in_maps(q, k, v, w_q, w_k, w_v, w_o):
    q = np.asarray(q, dtype=np.float32)
    k = np.asarray(k, dtype=np.float32)
    v = np.asarray(v, dtype=np.float32)
    w_q = np.asarray(w_q, dtype=np.float32)
    w_k = np.asarray(w_k, dtype=np.float32)
    w_v = np.asarray(w_v, dtype=np.float32)
    w_o = np.asarray(w_o, dtype=np.float32)
    B = q.shape[0]
    xqT = [np.ascontiguousarray(q[b].T) for b in range(B)]
    xkT = [np.ascontiguousarray(k[b].T) for b in range(B)]
    xvT = [np.ascontiguousarray(v[b].T) for b in range(B)]
    wqT = [np.ascontiguousarray(w_q[g * DG:(g + 1) * DG, :].T) for g in range(2)]
    wkT = [np.ascontiguousarray(w_k[g * DG:(g + 1) * DG, :].T) for g in range(2)]
    wvT = [np.ascontiguousarray(w_v[g * DG:(g + 1) * DG, :].T) for g in range(2)]
    woT = [np.ascontiguousarray(w_o[:, g * DG:(g + 1) * DG].T) for g in range(2)]
    sel8 = np.zeros((32, P), dtype=np.float32)
    for dt in range(4):
        sel8[dt * 8 + 2 * dt, 0:64] = 1.0
        sel8[dt * 8 + 2 * dt + 1, 64:128] = 1.0
    in_maps = []
    for c in range(8):
        b, g = c // 2, c % 2
        in_maps.append({
            "xqT": xqT[b], "xkT": xkT[b], "xvT": xvT[b],
            "wqT": wqT[g], "wkT": wkT[g], "wvT": wvT[g], "woT": woT[g],
            "sel8": sel8,
        })
    return in_maps


def kernel(q, k, v, mask, w_q, w_k, w_v, w_o):
    if "nc" not in _CACHE:
        _CACHE["nc"] = build_nc()
    nc = _CACHE["nc"]
    in_maps = make_in_maps(q, k, v, w_q, w_k, w_v, w_o)
    res = run_bass_kernel_spmd(nc, in_maps, list(range(8)))
    _CACHE["last_results"] = res
    B = np.asarray(q).shape[0]
    out = np.empty((B, L, DM), dtype=np.float32)
    for b in range(B):
        out[b] = (res.results[2 * b]["y"] + res.results[2 * b + 1]["y"]).T
    return out



# revision 14
# speedup vs baseline: 1.4541x; 1.4541x over previous
"""Linear-attention MultiHeadAttentionBlock kernel for 8 Trainium2 NeuronCores.

Sharding: core c handles (batch b = c//2, head-group g = c%2).  Each core
computes, for its batch's q/k/v and its 8 heads (512 of the 1024 d_model
dims):
    QfT = elu(Wq_g @ X_q^T)+1          (transposed space: d' on partitions)
    Kf  = elu(X_k @ Wk_g^T)+1          (n-space)
    ksum= ones^T-stationary accumulate over n-tiles -> (2, 512) -> 4 PE
          transposes -> ksumS[dt] (128, 8) masked stationaries
    Vp  = X_v @ Wv_g^T                 (n-space)
    KV  = per head-pair dt: Kf_dt^T @ Vp_dt (128x128, PE-accumulated over n)
    Zpre= ksumS[dt]-stationary matmuls vs QfT -> (8, L) head-major
    zr  = 1/Zpre (DVE), broadcast to (128, L) per dt on GpSimd,
          QfT *= zr (DVE, in place)
    kvcat = KV * block-diag mask (DVE)
    outT = kvcat_dt^T @ QfTz           (m'-space)
    y    = WoS @ out_g^T               (partial d_model-1024 output, bf16)
Host upcasts and sums the two per-batch partials.

All matmul operands are bf16 (halves HBM traffic vs fp32; PE row rate is
identical to fp32r).  PSUM accumulation stays fp32.
"""

import numpy as np

import concourse.bass as bass
import concourse.mybir as mybir
import concourse.tile as tile
from concourse import bacc
from concourse.bass_utils import run_bass_kernel_spmd
from concourse.masks import make_identity

P = 128
L = 2048          # sequence length
DM = 1024         # d_model (= contraction dim of projections)
DG = 512          # per-core head-group width (8 heads x 64)
NT = L // P       # 16 n-tiles
KC = DM // P      # 8 contraction chunks
DT = DG // P      # 4 d'-tiles (2 heads each)
NCH = 4           # n-chunks of 512
F32 = mybir.dt.float32
BF16 = mybir.dt.bfloat16

_CACHE = {}


def build_nc(repeats=1):
    nc = bacc.Bacc(None, target_bir_lowering=False)

    xq_d = nc.dram_tensor("xqT", [DM, L], BF16, kind="ExternalInput")
    xk_d = nc.dram_tensor("xkT", [DM, L], BF16, kind="ExternalInput")
    xv_d = nc.dram_tensor("xvT", [DM, L], BF16, kind="ExternalInput")
    wq_d = nc.dram_tensor("wqT", [DM, DG], BF16, kind="ExternalInput")
    wk_d = nc.dram_tensor("wkT", [DM, DG], BF16, kind="ExternalInput")
    wv_d = nc.dram_tensor("wvT", [DM, DG], BF16, kind="ExternalInput")
    wo_d = nc.dram_tensor("woT", [DG, DM], BF16, kind="ExternalInput")
    sel_d = nc.dram_tensor("sel8", [32, P], mybir.dt.float32r,
                           kind="ExternalInput")
    y_d = nc.dram_tensor("y", [DM, L], BF16, kind="ExternalOutput")

    with tile.TileContext(nc) as tc:
        with (
            tc.tile_pool(name="const", bufs=1) as cpool,
            tc.tile_pool(name="xt", bufs=17) as xt,      # (128,2048) input chunks
            tc.tile_pool(name="wt", bufs=8) as wt,       # (128,512) weight chunks
            tc.tile_pool(name="wo", bufs=4) as wop,      # (128,1024) w_o chunks
            tc.tile_pool(name="qft", bufs=16) as qftp,   # QfT persistent
            tc.tile_pool(name="kf", bufs=16) as kfp,     # Kf, later outT
            tc.tile_pool(name="vp", bufs=3) as vpp,      # Vp rotating
            tc.tile_pool(name="tmp", bufs=3) as tmp,     # feature-map temps
            tc.tile_pool(name="misc", bufs=1) as misc,   # zrA/ksumS/kvcat/ksA_sb
            tc.tile_pool(name="ysb", bufs=2) as ysb,     # (128,2048) y row-batches
            tc.tile_pool(name="pp", bufs=4, space="PSUM") as pp,
            tc.tile_pool(name="kvp", bufs=4, space="PSUM") as kvp,
        ):
            ident = cpool.tile([P, P], F32, name="ident")
            make_identity(nc, ident[:])
            ones_f = cpool.tile([P, 2], F32, name="ones_f")
            nc.gpsimd.memset(ones_f[:], 1.0)
            ones2 = cpool.tile([P, 2], BF16, name="ones2")
            nc.vector.tensor_copy(ones2[:], ones_f[:])
            # block-diag (128,128) mask: 1 where (i<64)==(j<64)
            bm_f = cpool.tile([P, P], F32, name="bm_f")
            nc.gpsimd.memset(bm_f[:], 0.0)
            nc.gpsimd.memset(bm_f[0:64, 0:64], 1.0)
            nc.gpsimd.memset(bm_f[64:128, 64:128], 1.0)
            blkmask = cpool.tile([P, P], BF16, name="blkmask")
            nc.vector.tensor_copy(blkmask[:], bm_f[:])
            # sel8[dt]: (8,128) broadcast matrix, row 2dt+s has 1s in
            # columns [64s, 64s+64)
            sel8 = []
            for _dt in range(4):
                st = cpool.tile([8, P], mybir.dt.float32r, name=f"sel8_{_dt}")
                nc.sync.dma_start(st[:], sel_d[_dt * 8:(_dt + 1) * 8, :])
                sel8.append(st)
            for _rep in range(repeats):
                body(nc, tc, ident, ones2, blkmask, sel8,
                     xt, wt, wop, qftp, kfp, vpp, tmp, misc, ysb, pp, kvp,
                     xq_d, xk_d, xv_d, wq_d, wk_d, wv_d, wo_d, y_d)

    nc.compile()
    return nc


def body(nc, tc, ident, ones2, blkmask, sel8,
         xt, wt, wop, qftp, kfp, vpp, tmp, misc, ysb, pp, kvp,
         xq_d, xk_d, xv_d, wq_d, wk_d, wv_d, wo_d, y_d):
    Exp = mybir.ActivationFunctionType.Exp
    Relu = mybir.ActivationFunctionType.Relu
    Alu = mybir.AluOpType

    def feature_map(ps, dst):
        # dst = elu(ps)+1 = exp(min(ps,0)) + relu(ps)
        t0 = tmp.tile([P, 512], F32, tag="tmp", name="t0")
        t1 = tmp.tile([P, 512], F32, tag="tmp", name="t1")
        nc.vector.tensor_scalar(t0[:], ps[:], 0.0, None, Alu.min)
        nc.scalar.activation(t1[:], ps[:], Relu)
        nc.scalar.activation(dst[:], t0[:], Exp)
        nc.vector.tensor_tensor(dst[:], dst[:], t1[:], Alu.add)

    def dma_x(src_d, kc):
        t = xt.tile([P, L], BF16, tag="xt", name="xc")
        nc.sync.dma_start(t[:], src_d[kc * P:(kc + 1) * P, :])
        return t

    # ---------------- Phase Q: QfT (transposed space) ----------------
    wq, xq = [], []
    for kc in range(KC):
        xq.append(dma_x(xq_d, kc))
        wt_t = wt.tile([P, DG], BF16, tag="wt")
        nc.sync.dma_start(wt_t[:], wq_d[kc * P:(kc + 1) * P, :])
        wq.append(wt_t)
    qft = [None] * 16  # (128, 512) tiles: index dt*NCH + nch
    for dt in range(DT):
        ypool, ytag = ((pp, "pp") if dt % 2 == 0 else (kvp, "acc"))
        psq = [ypool.tile([P, 512], F32, tag=ytag, name=f"psq{_n}")
               for _n in range(NCH)]
        for kc in range(KC):
            for nch in range(NCH):
                nc.tensor.matmul(
                    psq[nch][:],
                    wq[kc][:, dt * P:(dt + 1) * P],
                    xq[kc][:, nch * 512:(nch + 1) * 512],
                    start=(kc == 0), stop=(kc == KC - 1),
                )
        for nch in range(NCH):
            qf = qftp.tile([P, 512], BF16, tag="qft")
            feature_map(psq[nch], qf)
            qft[dt * NCH + nch] = qf

    # ---------------- Phase K: Kf (n-space) + ksum ----------------
    wk, xk = [], []
    for kc in range(KC):
        xk.append(dma_x(xk_d, kc))
        wt_t = wt.tile([P, DG], BF16, tag="wt")
        nc.sync.dma_start(wt_t[:], wk_d[kc * P:(kc + 1) * P, :])
        wk.append(wt_t)
    kf = []
    ksA = kvp.tile([2, 512], F32, tag="acc", name="ksA")
    for nt in range(NT):
        ps = pp.tile([P, 512], F32, tag="pp")
        for kc in range(KC):
            nc.tensor.matmul(
                ps[:],
                xk[kc][:, nt * P:(nt + 1) * P],
                wk[kc][:],
                start=(kc == 0), stop=(kc == KC - 1),
            )
        kft = kfp.tile([P, 512], BF16, tag="kf")
        feature_map(ps, kft)
        kf.append(kft)
        # ksum accumulate: (2,512) += ones2^T @ Kf_nt   (tiny stationary)
        nc.tensor.matmul(
            ksA[:], ones2[:], kft[:],
            start=(nt == 0), stop=(nt == NT - 1),
        )

    # ksum -> d'-partition masked stationaries ksumS[dt] (128, 8) bf16
    ksA_sb = misc.tile([2, 512], F32, tag="ksA_sb", name="ksA_sb")
    nc.scalar.copy(ksA_sb[:], ksA[0:2, :])
    ksumS = []
    for dt in range(DT):
        ztp = pp.tile([P, 2], F32, tag="pp", name="ztp")
        nc.tensor.transpose(ztp[:], ksA_sb[0:2, dt * P:(dt + 1) * P],
                            ident[0:2, 0:2])
        ks = misc.tile([P, 8], BF16, tag="ksumS", name=f"ksumS{dt}", bufs=4)
        nc.gpsimd.memset(ks[:], 0.0)
        nc.scalar.copy(ks[0:64, 2 * dt:2 * dt + 1], ztp[0:64, 0:1])
        nc.scalar.copy(ks[64:128, 2 * dt + 1:2 * dt + 2], ztp[64:128, 0:1])
        ksumS.append(ks)

    # ---------------- Phase V: Vp + KV accumulation + Z chain ----------------
    wv, xv = [], []
    for kc in range(KC):
        xv.append(dma_x(xv_d, kc))
        wt_t = wt.tile([P, DG], BF16, tag="wt")
        nc.sync.dma_start(wt_t[:], wv_d[kc * P:(kc + 1) * P, :])
        wv.append(wt_t)
    wo = []
    for dc in range(DT):
        wo_t = wop.tile([P, DM], BF16, tag="wo", name="wo_t")
        nc.sync.dma_start(wo_t[:], wo_d[dc * P:(dc + 1) * P, :])
        wo.append(wo_t)

    kvt = [kvp.tile([P, P], F32, tag="acc", name=f"kvt{_dt}")
           for _dt in range(DT)]
    zp = [None] * NCH
    zrA = misc.tile([8, L], mybir.dt.float32r, tag="zrA", name="zrA")
    for nt in range(NT):
        ps = pp.tile([P, 512], F32, tag="pp")
        for kc in range(KC):
            nc.tensor.matmul(
                ps[:],
                xv[kc][:, nt * P:(nt + 1) * P],
                wv[kc][:],
                start=(kc == 0), stop=(kc == KC - 1),
            )
        vp_t = vpp.tile([P, 512], BF16, tag="vp")
        nc.scalar.copy(vp_t[:], ps[:])
        for dt in range(DT):
            nc.tensor.matmul(
                kvt[dt][:],
                kf[nt][:, dt * P:(dt + 1) * P],
                vp_t[:, dt * P:(dt + 1) * P],
                start=(nt == 0), stop=(nt == NT - 1),
            )
        # interleaved Z chain (inputs qft/ksumS are ready since phase Q/K)
        if nt < NCH:
            # Zpre group for n-chunk `nt`: accumulate 4 masked stationaries
            nch = nt
            zp[nch] = pp.tile([8, 512], F32, tag="pp", name=f"zp{nch}")
            for dt in range(DT):
                nc.tensor.matmul(
                    zp[nch][:], ksumS[dt][:],
                    qft[dt * NCH + nch][:],
                    start=(dt == 0), stop=(dt == DT - 1),
                )
        elif nt < 8:
            nch = nt - 4
            with nc.allow_low_precision(reason="zr in tf32 is plenty"):
                nc.vector.reciprocal(zrA[:, nch * 512:(nch + 1) * 512],
                                     zp[nch][:])
        elif nt >= 8:
            # zr broadcast (PE, f32r) + in-place QfT scale, 2 tiles per nt
            for idx in (2 * (nt - 8), 2 * (nt - 8) + 1):
                dt, nch = idx // NCH, idx % NCH
                zrp = pp.tile([P, 512], F32, tag="pp", name="zrp")
                nc.tensor.matmul(
                    zrp[:], sel8[dt][:],
                    zrA[:, nch * 512:(nch + 1) * 512],
                    start=True, stop=True,
                )
                qt = qft[dt * NCH + nch]
                nc.vector.tensor_tensor(qt[:], qt[:], zrp[:], Alu.mult)

    # kvcat[dt] = KV block-diagonal via mask (bf16 for the outT stationary)
    kvcat = []
    for dt in range(DT):
        kvc = misc.tile([P, P], BF16, tag="kvcat", bufs=4)
        nc.vector.tensor_tensor(kvc[:], kvt[dt][:], blkmask[:], Alu.mult)
        kvcat.append(kvc)

    # ---------------- transposed out ----------------
    # outT[(dt, nch)]: (128 m'-part, 512 n) = kvcat[dt]^T @ QfTz
    outT = [[None] * DT for _ in range(NCH)]
    for dt in range(DT):
        for nch in range(NCH):
            otp = pp.tile([P, 512], F32, tag="pp", name="otp")
            nc.tensor.matmul(
                otp[:], kvcat[dt][:], qft[dt * NCH + nch][:],
                start=True, stop=True,
            )
            oT = kfp.tile([P, 512], BF16, tag="kf", name="oT")
            if (nch + dt) % 2 == 0:
                nc.vector.tensor_copy(oT[:], otp[:])
            else:
                nc.scalar.copy(oT[:], otp[:])
            outT[nch][dt] = oT

    # ---------------- final projection ----------------
    # yT = WoS @ out_g^T: wo blocks stationary, reused across the 4 n-chunks
    for jb in range(8):
        ypool, ytag = ((kvp, "acc") if jb % 2 == 0 else (pp, "pp"))
        yps = [ypool.tile([P, 512], F32, tag=ytag, name=f"yp{_n}")
               for _n in range(NCH)]
        for dc in range(DT):
            for nch in range(NCH):
                nc.tensor.matmul(
                    yps[nch][:],
                    wo[dc][:, jb * P:(jb + 1) * P],
                    outT[nch][dc][:],
                    start=(dc == 0), stop=(dc == DT - 1),
                )
        yt = ysb.tile([P, L], BF16, tag="ysb", name="yt")
        for nch in range(NCH):
            if (jb + nch) % 2 == 0:
                nc.vector.tensor_copy(
                    yt[:, nch * 512:(nch + 1) * 512], yps[nch][:])
            else:
                nc.scalar.copy(
                    yt[:, nch * 512:(nch + 1) * 512], yps[nch][:])
            nc.sync.dma_start(
                y_d[jb * P:(jb + 1) * P, nch * 512:(nch + 1) * 512],
                yt[:, nch * 512:(nch + 1) * 512])


def make_in_maps(q, k, v, w_q, w_k, w_v, w_o):
    bf16 = mybir.dt.np(BF16)
    q = np.asarray(q, dtype=np.float32)
    k = np.asarray(k, dtype=np.float32)
    v = np.asarray(v, dtype=np.float32)
    w_q = np.asarray(w_q, dtype=np.float32)
    w_k = np.asarray(w_k, dtype=np.float32)
    w_v = np.asarray(w_v, dtype=np.float32)
    w_o = np.asarray(w_o, dtype=np.float32)
    B = q.shape[0]
    xqT = [np.ascontiguousarray(q[b].T).astype(bf16) for b in range(B)]
    xkT = [np.ascontiguousarray(k[b].T).astype(bf16) for b in range(B)]
    xvT = [np.ascontiguousarray(v[b].T).astype(bf16) for b in range(B)]
    wqT = [np.ascontiguousarray(w_q[g * DG:(g + 1) * DG, :].T).astype(bf16)
           for g in range(2)]
    wkT = [np.ascontiguousarray(w_k[g * DG:(g + 1) * DG, :].T).astype(bf16)
           for g in range(2)]
    wvT = [np.ascontiguousarray(w_v[g * DG:(g + 1) * DG, :].T).astype(bf16)
           for g in range(2)]
    woT = [np.ascontiguousarray(w_o[:, g * DG:(g + 1) * DG].T).astype(bf16)
           for g in range(2)]
    sel8 = np.zeros((32, P), dtype=np.float32)
    for dt in range(4):
        sel8[dt * 8 + 2 * dt, 0:64] = 1.0
        sel8[dt * 8 + 2 * dt + 1, 64:128] = 1.0
    in_maps = []
    for c in range(8):
        b, g = c // 2, c % 2
        in_maps.append({
            "xqT": xqT[b], "xkT": xkT[b], "xvT": xvT[b],
            "wqT": wqT[g], "wkT": wkT[g], "wvT": wvT[g], "woT": woT[g],
            "sel8": sel8,
        })
    return in_maps


def kernel(q, k, v, mask, w_q, w_k, w_v, w_o):
    if "nc" not in _CACHE:
        _CACHE["nc"] = build_nc()
    nc = _CACHE["nc"]
    in_maps = make_in_maps(q, k, v, w_q, w_k, w_v, w_o)
    res = run_bass_kernel_spmd(nc, in_maps, list(range(8)))
    _CACHE["last_results"] = res
    B = np.asarray(q).shape[0]
    out = np.empty((B, L, DM), dtype=np.float32)
    for b in range(B):
        out[b] = (res.results[2 * b]["y"].astype(np.float32)
                  + res.results[2 * b + 1]["y"].astype(np.float32)).T
    return out


# revision 30
# speedup vs baseline: 1.7363x; 1.1940x over previous
"""Linear-attention MultiHeadAttentionBlock kernel for 8 Trainium2 NeuronCores.

Sharding: core c handles (batch b = c//2, head-group g = c%2).  Each core
computes, for its batch's q/k/v and its 8 heads (512 of the 1024 d_model
dims):
    QfT = elu(Wq_g @ X_q^T)+1          (transposed space: d' on partitions)
    Kf  = elu(X_k @ Wk_g^T)+1          (n-space)
    ksum= ones^T-stationary accumulate over n-tiles -> (2, 512) -> 4 PE
          transposes -> ksumS[dt] (128, 8) masked stationaries
    Vp  = X_v @ Wv_g^T                 (n-space)
    KV  = per head-pair dt: Kf_dt^T @ Vp_dt (128x128, PE-accumulated over n)
    Zpre= ksumS[dt]-stationary matmuls vs QfT -> (8, L) head-major
    zr  = 1/Zpre (DVE), broadcast to (128, L) per dt on GpSimd,
          QfT *= zr (DVE, in place)
    kvcat = KV * block-diag mask (DVE)
    outT = kvcat_dt^T @ QfTz           (m'-space)
    y    = WoS @ out_g^T               (partial d_model-1024 output, bf16)
Host upcasts and sums the two per-batch partials.

All matmul operands are bf16 (halves HBM traffic vs fp32; PE row rate is
identical to fp32r).  PSUM accumulation stays fp32.
"""

import numpy as np

import concourse.bass as bass
import concourse.mybir as mybir
import concourse.tile as tile
from concourse import bacc
from concourse.bass_utils import run_bass_kernel_spmd
from concourse.masks import make_identity

P = 128
L = 2048          # sequence length
DM = 1024         # d_model (= contraction dim of projections)
DG = 512          # per-core head-group width (8 heads x 64)
NT = L // P       # 16 n-tiles
KC = DM // P      # 8 contraction chunks
DT = DG // P      # 4 d'-tiles (2 heads each)
NCH = 4           # n-chunks of 512
F32 = mybir.dt.float32
BF16 = mybir.dt.bfloat16

_CACHE = {}


def build_nc(repeats=1):
    nc = bacc.Bacc(None, target_bir_lowering=False)

    xq_d = nc.dram_tensor("xqT", [DM, L], BF16, kind="ExternalInput")
    xk_d = nc.dram_tensor("xkT", [DM, L], BF16, kind="ExternalInput")
    xv_d = nc.dram_tensor("xvT", [DM, L], BF16, kind="ExternalInput")
    wq_d = nc.dram_tensor("wqT", [DM, DG], BF16, kind="ExternalInput")
    wk_d = nc.dram_tensor("wkT", [DM, DG], BF16, kind="ExternalInput")
    wv_d = nc.dram_tensor("wvT", [DM, DG], BF16, kind="ExternalInput")
    wo_d = nc.dram_tensor("woT", [DG, DM], BF16, kind="ExternalInput")
    sel_d = nc.dram_tensor("sel8", [32, P], mybir.dt.float32r,
                           kind="ExternalInput")
    y_d = nc.dram_tensor("y", [DM, L], BF16, kind="ExternalOutput")

    with tile.TileContext(nc) as tc:
        with (
            tc.tile_pool(name="const", bufs=1) as cpool,
            tc.tile_pool(name="xt", bufs=2) as xt,       # (128,8,2048) x tensors
            tc.tile_pool(name="wt", bufs=2) as wt,       # (128,8,512) weights
            tc.tile_pool(name="wo", bufs=1) as wop,      # (128,4,1024) w_o
            tc.tile_pool(name="qft", bufs=16) as qftp,   # QfT persistent
            tc.tile_pool(name="kf", bufs=16) as kfp,     # Kf, later outT
            tc.tile_pool(name="vp", bufs=3) as vpp,      # Vp rotating
            tc.tile_pool(name="tmp", bufs=3) as tmp,     # feature-map temps
            tc.tile_pool(name="misc", bufs=1) as misc,   # zrA/ksumS/kvcat/ksA_sb
            tc.tile_pool(name="ysb", bufs=2) as ysb,     # (128,2048) y row-batches
            tc.tile_pool(name="pp", bufs=4, space="PSUM") as pp,
            tc.tile_pool(name="kvp", bufs=4, space="PSUM") as kvp,
        ):
            ident = cpool.tile([P, P], F32, name="ident")
            make_identity(nc, ident[:])
            ones_f = cpool.tile([P, 2], F32, name="ones_f")
            nc.gpsimd.memset(ones_f[:], 1.0)
            ones2 = cpool.tile([P, 2], BF16, name="ones2")
            nc.vector.tensor_copy(ones2[:], ones_f[:])
            # block-diag (128,128) mask: 1 where (i<64)==(j<64)
            bm_f = cpool.tile([P, P], F32, name="bm_f")
            nc.gpsimd.memset(bm_f[:], 0.0)
            nc.gpsimd.memset(bm_f[0:64, 0:64], 1.0)
            nc.gpsimd.memset(bm_f[64:128, 64:128], 1.0)
            blkmask = cpool.tile([P, P], BF16, name="blkmask")
            nc.vector.tensor_copy(blkmask[:], bm_f[:])
            for _rep in range(repeats):
                body(nc, tc, ident, ones2, blkmask, cpool,
                     xt, wt, wop, qftp, kfp, vpp, tmp, misc, ysb, pp, kvp,
                     xq_d, xk_d, xv_d, wq_d, wk_d, wv_d, wo_d, sel_d, y_d)

    nc.compile()
    return nc


def body(nc, tc, ident, ones2, blkmask, cpool,
         xt, wt, wop, qftp, kfp, vpp, tmp, misc, ysb, pp, kvp,
         xq_d, xk_d, xv_d, wq_d, wk_d, wv_d, wo_d, sel_d, y_d):
    Exp = mybir.ActivationFunctionType.Exp
    Relu = mybir.ActivationFunctionType.Relu
    Alu = mybir.AluOpType

    def feature_map(ps, dst):
        # dst = elu(ps)+1 = exp(min(ps,0)) + relu(ps)
        # (spread over DVE/ACT/ACT/Pool to keep per-engine queues short)
        t0 = tmp.tile([P, 512], F32, tag="tmp", name="t0")
        t1 = tmp.tile([P, 512], F32, tag="tmp", name="t1")
        nc.vector.tensor_scalar(t0[:], ps[:], 0.0, None, Alu.min)
        nc.scalar.activation(t1[:], ps[:], Relu)
        nc.scalar.activation(dst[:], t0[:], Exp)
        nc.gpsimd.tensor_tensor(dst[:], dst[:], t1[:], Alu.add)

    def dma_x_all(src_d, name, npieces=1):
        # all 8 chunks as one (128, 8, L) tile; optionally split the DMA
        # into kc-groups so early chunks land before the full transfer
        t = xt.tile([P, KC, L], BF16, tag="xa", name=name, bufs=2)
        src = src_d.rearrange("(c p) n -> p c n", p=P)
        step = KC // npieces
        for i in range(0, KC, step):
            nc.sync.dma_start(t[:, i:i + step, :], src[:, i:i + step, :])
        return t

    def dma_w_all(src_d, name, npieces=1):
        # all 8 weight chunks as one (128, 8, DG) tile
        t = wt.tile([P, KC, DG], BF16, tag="wt", name=name, bufs=2)
        src = src_d.rearrange("(c p) n -> p c n", p=P)
        step = KC // npieces
        for i in range(0, KC, step):
            nc.sync.dma_start(t[:, i:i + step, :], src[:, i:i + step, :])
        return t

    # ---------------- Phase Q: QfT (transposed space) ----------------
    # split + interleave the first x/w transfers so chunk kc=0 lands quickly
    xqa = xt.tile([P, KC, L], BF16, tag="xa", name="xq", bufs=2)
    wqa = wt.tile([P, KC, DG], BF16, tag="wt", name="wq", bufs=2)
    xq_src = xq_d.rearrange("(c p) n -> p c n", p=P)
    wq_src = wq_d.rearrange("(c p) n -> p c n", p=P)
    nc.sync.dma_start(xqa[:, 0:1, :], xq_src[:, 0:1, :])
    nc.sync.dma_start(wqa[:, 0:2, :], wq_src[:, 0:2, :])
    nc.sync.dma_start(xqa[:, 1:2, :], xq_src[:, 1:2, :])
    nc.sync.dma_start(wqa[:, 2:8, :], wq_src[:, 2:8, :])
    nc.sync.dma_start(xqa[:, 2:4, :], xq_src[:, 2:4, :])
    nc.sync.dma_start(xqa[:, 4:6, :], xq_src[:, 4:6, :])
    nc.sync.dma_start(xqa[:, 6:8, :], xq_src[:, 6:8, :])

    qft = [None] * 16  # (128, 512) tiles: index dt*NCH + nch
    for dt in range(DT):
        ypool, ytag = ((pp, "pp") if dt % 2 == 0 else (kvp, "acc"))
        psq = [ypool.tile([P, 512], F32, tag=ytag, name=f"psq{_n}")
               for _n in range(NCH)]
        for kc in range(KC):
            for nch in range(NCH):
                nc.tensor.matmul(
                    psq[nch][:],
                    wqa[:, kc, dt * P:(dt + 1) * P],
                    xqa[:, kc, nch * 512:(nch + 1) * 512],
                    start=(kc == 0), stop=(kc == KC - 1),
                )
        for nch in range(NCH):
            qf = qftp.tile([P, 512], BF16, tag="qft")
            feature_map(psq[nch], qf)
            qft[dt * NCH + nch] = qf

    # ---------------- Phase K: Kf (n-space) + ksum ----------------
    xka = dma_x_all(xk_d, "xk")
    wka = dma_w_all(wk_d, "wk")
    kf = []
    ksA = kvp.tile([2, 512], F32, tag="acc", name="ksA")
    for nt in range(NT):
        ps = pp.tile([P, 512], F32, tag="pp")
        for kc in range(KC):
            nc.tensor.matmul(
                ps[:],
                xka[:, kc, nt * P:(nt + 1) * P],
                wka[:, kc, :],
                start=(kc == 0), stop=(kc == KC - 1),
            )
        kft = kfp.tile([P, 512], BF16, tag="kf")
        feature_map(ps, kft)
        kf.append(kft)
        # ksum accumulate: (2,512) += ones2^T @ Kf_(nt-1), staggered one
        # n-tile behind the projections so the PE never waits on the
        # feature-map chain
        if nt > 0:
            nc.tensor.matmul(
                ksA[:], ones2[:], kf[nt - 1][:],
                start=(nt == 1), stop=False,
            )

    def ksum_tail():
        # last ksA accumulate + ksum -> d'-partition masked stationaries
        # ksumS[dt] (128, 8) bf16.  Emitted early in phase V so the PE is
        # never parked on kf[15]'s feature-map chain.
        nc.tensor.matmul(ksA[:], ones2[:], kf[NT - 1][:],
                         start=False, stop=True)
        ksA_sb = misc.tile([2, 512], F32, tag="ksA_sb", name="ksA_sb")
        nc.scalar.copy(ksA_sb[:], ksA[0:2, :])
        ksumS = []
        for dt in range(DT):
            ztp = pp.tile([P, 2], F32, tag="pp", name="ztp")
            nc.tensor.transpose(ztp[:], ksA_sb[0:2, dt * P:(dt + 1) * P],
                                ident[0:2, 0:2])
            ks = misc.tile([P, 8], BF16, tag="ksumS", name=f"ksumS{dt}",
                           bufs=4)
            nc.gpsimd.memset(ks[:], 0.0)
            nc.scalar.copy(ks[0:64, 2 * dt:2 * dt + 1], ztp[0:64, 0:1])
            nc.scalar.copy(ks[64:128, 2 * dt + 1:2 * dt + 2],
                           ztp[64:128, 0:1])
            ksumS.append(ks)
        return ksumS

    # ---------------- Phase V: Vp + KV accumulation + Z chain ----------------
    xva = dma_x_all(xv_d, "xv")
    wva = dma_w_all(wv_d, "wv")
    woa = wop.tile([P, DT, DM], BF16, tag="wo", name="wo_t", bufs=1)
    nc.sync.dma_start(woa[:], wo_d.rearrange("(c p) n -> p c n", p=P))
    # sel8[dt]: (8,128) broadcast matrix, row 2dt+s has 1s in cols [64s,64s+64)
    sel8t = cpool.tile([8, 4, P], mybir.dt.float32r, name="sel8t")
    nc.sync.dma_start(sel8t[:], sel_d.rearrange("(d s) n -> s d n", s=8))
    sel8 = [sel8t[:, _dt, :] for _dt in range(4)]

    kvt = [kvp.tile([P, P], F32, tag="acc", name=f"kvt{_dt}")
           for _dt in range(DT)]
    zp = [None] * NCH
    zrA = misc.tile([8, L], mybir.dt.float32r, tag="zrA", name="zrA")

    def kv_mms(nt):
        vt, kft = vps[nt % 3], kf[nt]
        for dt in range(DT):
            nc.tensor.matmul(
                kvt[dt][:],
                kft[:, dt * P:(dt + 1) * P],
                vt[:, dt * P:(dt + 1) * P],
                start=(nt == 0), stop=(nt == NT - 1),
            )

    vps = [None] * 3
    for nt in range(NT):
        ps = pp.tile([P, 512], F32, tag="pp")
        for kc in range(KC):
            nc.tensor.matmul(
                ps[:],
                xva[:, kc, nt * P:(nt + 1) * P],
                wva[:, kc, :],
                start=(kc == 0), stop=(kc == KC - 1),
            )
        vp_t = vpp.tile([P, 512], BF16, tag="vp")
        nc.scalar.copy(vp_t[:], ps[:])
        vps[nt % 3] = vp_t
        if nt == 0:
            ksumS = ksum_tail()
        # KV matmuls staggered one n-tile behind the projections so the PE
        # never waits on the Vp copy
        if nt > 0:
            kv_mms(nt - 1)
        # interleaved Z chain (inputs qft/ksumS ready since phase Q/K)
        if 2 <= nt < 6:
            # Zpre group for n-chunk nt-2: accumulate 4 masked stationaries
            nch = nt - 2
            zp[nch] = pp.tile([8, 512], F32, tag="pp", name=f"zp{nch}")
            for dt in range(DT):
                nc.tensor.matmul(
                    zp[nch][:], ksumS[dt][:],
                    qft[dt * NCH + nch][:],
                    start=(dt == 0), stop=(dt == DT - 1),
                )
        elif 6 <= nt < 10:
            nch = nt - 6
            with nc.allow_low_precision(reason="zr in tf32 is plenty"):
                nc.vector.reciprocal(zrA[:, nch * 512:(nch + 1) * 512],
                                     zp[nch][:])
        elif nt >= 10:
            # zr broadcast (PE, f32r) + in-place QfT scale
            lo, hi = 3 * (nt - 10), min(16, 3 * (nt - 10) + 3)
            for idx in range(lo, hi):
                dt, nch = idx // NCH, idx % NCH
                zrp = pp.tile([P, 512], F32, tag="pp", name="zrp")
                nc.tensor.matmul(
                    zrp[:], sel8[dt][:],
                    zrA[:, nch * 512:(nch + 1) * 512],
                    start=True, stop=True,
                )
                qt = qft[dt * NCH + nch]
                nc.vector.tensor_tensor(qt[:], qt[:], zrp[:], Alu.mult)
    kv_mms(NT - 1)

    # kvcat[dt] = KV block-diagonal via mask (bf16 for the outT stationary)
    kvcat = []
    for dt in range(DT):
        kvc = misc.tile([P, P], BF16, tag="kvcat", bufs=4)
        nc.vector.tensor_tensor(kvc[:], kvt[dt][:], blkmask[:], Alu.mult)
        kvcat.append(kvc)

    # ---------------- transposed out ----------------
    # outT[(dt, nch)]: (128 m'-part, 512 n) = kvcat[dt]^T @ QfTz
    outT = [[None] * DT for _ in range(NCH)]
    for dt in range(DT):
        for nch in range(NCH):
            otp = pp.tile([P, 512], F32, tag="pp", name="otp")
            nc.tensor.matmul(
                otp[:], kvcat[dt][:], qft[dt * NCH + nch][:],
                start=True, stop=True,
            )
            oT = kfp.tile([P, 512], BF16, tag="kf", name="oT")
            if (nch + dt) % 2 == 0:
                nc.vector.tensor_copy(oT[:], otp[:])
            else:
                nc.scalar.copy(oT[:], otp[:])
            outT[nch][dt] = oT

    # ---------------- final projection ----------------
    # yT = WoS @ out_g^T: wo blocks stationary, reused across the 4 n-chunks
    for jb in range(8):
        ypool, ytag = ((kvp, "acc") if jb % 2 == 0 else (pp, "pp"))
        yps = [ypool.tile([P, 512], F32, tag=ytag, name=f"yp{_n}")
               for _n in range(NCH)]
        for dc in range(DT):
            for nch in range(NCH):
                nc.tensor.matmul(
                    yps[nch][:],
                    woa[:, dc, jb * P:(jb + 1) * P],
                    outT[nch][dc][:],
                    start=(dc == 0), stop=(dc == DT - 1),
                )
        yt = ysb.tile([P, L], BF16, tag="ysb", name="yt")
        for nch in range(NCH):
            if (jb + nch) % 2 == 0:
                nc.vector.tensor_copy(
                    yt[:, nch * 512:(nch + 1) * 512], yps[nch][:])
            else:
                nc.scalar.copy(
                    yt[:, nch * 512:(nch + 1) * 512], yps[nch][:])
        nc.sync.dma_start(y_d[jb * P:(jb + 1) * P, :], yt[:])


def make_in_maps(q, k, v, w_q, w_k, w_v, w_o):
    bf16 = mybir.dt.np(BF16)
    q = np.asarray(q, dtype=np.float32)
    k = np.asarray(k, dtype=np.float32)
    v = np.asarray(v, dtype=np.float32)
    w_q = np.asarray(w_q, dtype=np.float32)
    w_k = np.asarray(w_k, dtype=np.float32)
    w_v = np.asarray(w_v, dtype=np.float32)
    w_o = np.asarray(w_o, dtype=np.float32)
    B = q.shape[0]
    xqT = [np.ascontiguousarray(q[b].T).astype(bf16) for b in range(B)]
    xkT = [np.ascontiguousarray(k[b].T).astype(bf16) for b in range(B)]
    xvT = [np.ascontiguousarray(v[b].T).astype(bf16) for b in range(B)]
    wqT = [np.ascontiguousarray(w_q[g * DG:(g + 1) * DG, :].T).astype(bf16)
           for g in range(2)]
    wkT = [np.ascontiguousarray(w_k[g * DG:(g + 1) * DG, :].T).astype(bf16)
           for g in range(2)]
    wvT = [np.ascontiguousarray(w_v[g * DG:(g + 1) * DG, :].T).astype(bf16)
           for g in range(2)]
    woT = [np.ascontiguousarray(w_o[:, g * DG:(g + 1) * DG].T).astype(bf16)
           for g in range(2)]
    sel8 = np.zeros((32, P), dtype=np.float32)
    for dt in range(4):
        sel8[dt * 8 + 2 * dt, 0:64] = 1.0
        sel8[dt * 8 + 2 * dt + 1, 64:128] = 1.0
    in_maps = []
    for c in range(8):
        b, g = c // 2, c % 2
        in_maps.append({
            "xqT": xqT[b], "xkT": xkT[b], "xvT": xvT[b],
            "wqT": wqT[g], "wkT": wkT[g], "wvT": wvT[g], "woT": woT[g],
            "sel8": sel8,
        })
    return in_maps


def kernel(q, k, v, mask, w_q, w_k, w_v, w_o):
    if "nc" not in _CACHE:
        _CACHE["nc"] = build_nc()
    nc = _CACHE["nc"]
    in_maps = make_in_maps(q, k, v, w_q, w_k, w_v, w_o)
    res = run_bass_kernel_spmd(nc, in_maps, list(range(8)))
    _CACHE["last_results"] = res
    B = np.asarray(q).shape[0]
    out = np.empty((B, L, DM), dtype=np.float32)
    for b in range(B):
        out[b] = (res.results[2 * b]["y"].astype(np.float32)
                  + res.results[2 * b + 1]["y"].astype(np.float32)).T
    return out


# revision 33
# speedup vs baseline: 1.8700x; 1.0770x over previous
"""Linear-attention MultiHeadAttentionBlock kernel for 8 Trainium2 NeuronCores.

Sharding: core c handles (batch b = c//2, head-group g = c%2).  Each core
computes, for its batch's q/k/v and its 8 heads (512 of the 1024 d_model
dims):
    QfT = elu(Wq_g @ X_q^T)+1          (transposed space: d' on partitions)
    Kf  = elu(X_k @ Wk_g^T)+1          (n-space)
    ksum= ones^T-stationary accumulate over n-tiles -> (2, 512) -> 4 PE
          transposes -> ksumS[dt] (128, 8) masked stationaries
    Vp  = X_v @ Wv_g^T                 (n-space)
    KV  = per head-pair dt: Kf_dt^T @ Vp_dt (128x128, PE-accumulated over n)
    Zpre= ksumS[dt]-stationary matmuls vs QfT -> (8, L) head-major
    zr  = 1/Zpre (DVE), broadcast to (128, L) per dt on GpSimd,
          QfT *= zr (DVE, in place)
    kvcat = KV * block-diag mask (DVE)
    outT = kvcat_dt^T @ QfTz           (m'-space)
    y    = WoS @ out_g^T               (partial d_model-1024 output, bf16)
Host upcasts and sums the two per-batch partials.

All matmul operands are bf16 (halves HBM traffic vs fp32; PE row rate is
identical to fp32r).  PSUM accumulation stays fp32.
"""

import numpy as np

import concourse.bass as bass
import concourse.mybir as mybir
import concourse.tile as tile
from concourse import bacc
from concourse.bass_utils import run_bass_kernel_spmd
from concourse.masks import make_identity

P = 128
L = 2048          # sequence length
DM = 1024         # d_model (= contraction dim of projections)
DG = 512          # per-core head-group width (8 heads x 64)
NT = L // P       # 16 n-tiles
KC = DM // P      # 8 contraction chunks
DT = DG // P      # 4 d'-tiles (2 heads each)
NCH = 4           # n-chunks of 512
F32 = mybir.dt.float32
BF16 = mybir.dt.bfloat16

_CACHE = {}


def build_nc(repeats=1):
    nc = bacc.Bacc(None, target_bir_lowering=False)

    xq_d = nc.dram_tensor("xqT", [DM, L], BF16, kind="ExternalInput")
    xk_d = nc.dram_tensor("xkT", [DM, L], BF16, kind="ExternalInput")
    xv_d = nc.dram_tensor("xvT", [DM, L], BF16, kind="ExternalInput")
    wq_d = nc.dram_tensor("wqT", [DM, DG], BF16, kind="ExternalInput")
    wk_d = nc.dram_tensor("wkT", [DM, DG], BF16, kind="ExternalInput")
    wv_d = nc.dram_tensor("wvT", [DM, DG], BF16, kind="ExternalInput")
    wo_d = nc.dram_tensor("woT", [DG, DM], BF16, kind="ExternalInput")
    sel_d = nc.dram_tensor("sel8", [32, P], mybir.dt.float32r,
                           kind="ExternalInput")
    y_d = nc.dram_tensor("y", [DM, L], BF16, kind="ExternalOutput")

    with tile.TileContext(nc) as tc:
        with (
            tc.tile_pool(name="const", bufs=1) as cpool,
            tc.tile_pool(name="xt", bufs=2) as xt,       # (128,8,2048) x tensors
            tc.tile_pool(name="wt", bufs=2) as wt,       # (128,8,512) weights
            tc.tile_pool(name="wo", bufs=1) as wop,      # (128,4,1024) w_o
            tc.tile_pool(name="qft", bufs=16) as qftp,   # QfT persistent
            tc.tile_pool(name="kf", bufs=16) as kfp,     # Kf, later outT
            tc.tile_pool(name="vp", bufs=3) as vpp,      # Vp rotating
            tc.tile_pool(name="tmp", bufs=8) as tmp,     # feature-map temps
            tc.tile_pool(name="misc", bufs=1) as misc,   # zrA/ksumS/kvcat/ksA_sb
            tc.tile_pool(name="ysb", bufs=2) as ysb,     # (128,2048) y row-batches
            tc.tile_pool(name="pp", bufs=4, space="PSUM") as pp,
            tc.tile_pool(name="kvp", bufs=4, space="PSUM") as kvp,
        ):
            ident = cpool.tile([P, P], F32, name="ident")
            make_identity(nc, ident[:])
            ones_f = cpool.tile([P, 2], F32, name="ones_f")
            nc.gpsimd.memset(ones_f[:], 1.0)
            ones2 = cpool.tile([P, 2], BF16, name="ones2")
            nc.vector.tensor_copy(ones2[:], ones_f[:])
            # block-diag (128,128) mask: 1 where (i<64)==(j<64)
            bm_f = cpool.tile([P, P], F32, name="bm_f")
            nc.gpsimd.memset(bm_f[:], 0.0)
            nc.gpsimd.memset(bm_f[0:64, 0:64], 1.0)
            nc.gpsimd.memset(bm_f[64:128, 64:128], 1.0)
            blkmask = cpool.tile([P, P], BF16, name="blkmask")
            nc.vector.tensor_copy(blkmask[:], bm_f[:])
            for _rep in range(repeats):
                body(nc, tc, ident, ones2, blkmask, cpool,
                     xt, wt, wop, qftp, kfp, vpp, tmp, misc, ysb, pp, kvp,
                     xq_d, xk_d, xv_d, wq_d, wk_d, wv_d, wo_d, sel_d, y_d)

    nc.compile()
    return nc


def body(nc, tc, ident, ones2, blkmask, cpool,
         xt, wt, wop, qftp, kfp, vpp, tmp, misc, ysb, pp, kvp,
         xq_d, xk_d, xv_d, wq_d, wk_d, wv_d, wo_d, sel_d, y_d):
    Exp = mybir.ActivationFunctionType.Exp
    Relu = mybir.ActivationFunctionType.Relu
    Alu = mybir.AluOpType

    def feature_map(ps, dst):
        # dst = elu(ps)+1 = exp(min(ps,0)) + relu(ps)
        # (spread over DVE/ACT/ACT/Pool to keep per-engine queues short)
        t0 = tmp.tile([P, 512], F32, tag="tmp", name="t0")
        t1 = tmp.tile([P, 512], F32, tag="tmp", name="t1")
        nc.vector.tensor_scalar(t0[:], ps[:], 0.0, None, Alu.min)
        nc.scalar.activation(t1[:], ps[:], Relu)
        nc.scalar.activation(dst[:], t0[:], Exp)
        nc.vector.tensor_tensor(dst[:], dst[:], t1[:], Alu.add)

    def dma_x_all(src_d, name, npieces=1):
        # all 8 chunks as one (128, 8, L) tile; optionally split the DMA
        # into kc-groups so early chunks land before the full transfer
        t = xt.tile([P, KC, L], BF16, tag="xa", name=name, bufs=2)
        src = src_d.rearrange("(c p) n -> p c n", p=P)
        step = KC // npieces
        for i in range(0, KC, step):
            nc.sync.dma_start(t[:, i:i + step, :], src[:, i:i + step, :])
        return t

    def dma_w_all(src_d, name, npieces=1):
        # all 8 weight chunks as one (128, 8, DG) tile
        t = wt.tile([P, KC, DG], BF16, tag="wt", name=name, bufs=2)
        src = src_d.rearrange("(c p) n -> p c n", p=P)
        step = KC // npieces
        for i in range(0, KC, step):
            nc.sync.dma_start(t[:, i:i + step, :], src[:, i:i + step, :])
        return t

    # ---------------- Phase Q: QfT (transposed space) ----------------
    # split + interleave the first x/w transfers so chunk kc=0 lands quickly
    xqa = xt.tile([P, KC, L], BF16, tag="xa", name="xq", bufs=2)
    wqa = wt.tile([P, KC, DG], BF16, tag="wt", name="wq", bufs=2)
    xq_src = xq_d.rearrange("(c p) n -> p c n", p=P)
    wq_src = wq_d.rearrange("(c p) n -> p c n", p=P)
    nc.sync.dma_start(xqa[:, 0:1, :], xq_src[:, 0:1, :])
    nc.sync.dma_start(wqa[:, 0:2, :], wq_src[:, 0:2, :])
    nc.sync.dma_start(xqa[:, 1:2, :], xq_src[:, 1:2, :])
    nc.sync.dma_start(wqa[:, 2:8, :], wq_src[:, 2:8, :])
    nc.sync.dma_start(xqa[:, 2:4, :], xq_src[:, 2:4, :])
    nc.sync.dma_start(xqa[:, 4:6, :], xq_src[:, 4:6, :])
    nc.sync.dma_start(xqa[:, 6:8, :], xq_src[:, 6:8, :])

    qft = [None] * 16  # (128, 512) tiles: index dt*NCH + nch
    for dt in range(DT):
        ypool, ytag = ((pp, "pp") if dt % 2 == 0 else (kvp, "acc"))
        psq = [ypool.tile([P, 512], F32, tag=ytag, name=f"psq{_n}")
               for _n in range(NCH)]
        for kc in range(KC):
            for nch in range(NCH):
                nc.tensor.matmul(
                    psq[nch][:],
                    wqa[:, kc, dt * P:(dt + 1) * P],
                    xqa[:, kc, nch * 512:(nch + 1) * 512],
                    start=(kc == 0), stop=(kc == KC - 1),
                )
        for nch in range(NCH):
            qf = qftp.tile([P, 512], BF16, tag="qft")
            feature_map(psq[nch], qf)
            qft[dt * NCH + nch] = qf

    # ---------------- Phase K: Kf (n-space) + ksum ----------------
    xka = dma_x_all(xk_d, "xk")
    wka = dma_w_all(wk_d, "wk")
    kf = []
    ksA = kvp.tile([2, 512], F32, tag="acc", name="ksA")
    for nt in range(NT):
        ps = pp.tile([P, 512], F32, tag="pp")
        for kc in range(KC):
            nc.tensor.matmul(
                ps[:],
                xka[:, kc, nt * P:(nt + 1) * P],
                wka[:, kc, :],
                start=(kc == 0), stop=(kc == KC - 1),
            )
        kft = kfp.tile([P, 512], BF16, tag="kf")
        feature_map(ps, kft)
        kf.append(kft)
        # ksum accumulate: (2,512) += ones2^T @ Kf_(nt-1), staggered one
        # n-tile behind the projections so the PE never waits on the
        # feature-map chain
        if nt > 0:
            nc.tensor.matmul(
                ksA[:], ones2[:], kf[nt - 1][:],
                start=(nt == 1), stop=False,
            )

    def ksum_tail():
        # last ksA accumulate + ksum -> d'-partition masked stationaries
        # ksumS[dt] (128, 8) bf16.  Emitted early in phase V so the PE is
        # never parked on kf[15]'s feature-map chain.
        nc.tensor.matmul(ksA[:], ones2[:], kf[NT - 1][:],
                         start=False, stop=True)
        ksA_sb = misc.tile([2, 512], F32, tag="ksA_sb", name="ksA_sb")
        nc.scalar.copy(ksA_sb[:], ksA[0:2, :])
        ksumS = []
        for dt in range(DT):
            ztp = pp.tile([P, 2], F32, tag="pp", name="ztp")
            nc.tensor.transpose(ztp[:], ksA_sb[0:2, dt * P:(dt + 1) * P],
                                ident[0:2, 0:2])
            ks = misc.tile([P, 8], BF16, tag="ksumS", name=f"ksumS{dt}",
                           bufs=4)
            nc.gpsimd.memset(ks[:], 0.0)
            nc.scalar.copy(ks[0:64, 2 * dt:2 * dt + 1], ztp[0:64, 0:1])
            nc.scalar.copy(ks[64:128, 2 * dt + 1:2 * dt + 2],
                           ztp[64:128, 0:1])
            ksumS.append(ks)
        return ksumS

    # ---------------- Phase V: Vp + KV accumulation + Z chain ----------------
    xva = dma_x_all(xv_d, "xv")
    wva = dma_w_all(wv_d, "wv")
    woa = wop.tile([P, DT, DM], BF16, tag="wo", name="wo_t", bufs=1)
    nc.sync.dma_start(woa[:], wo_d.rearrange("(c p) n -> p c n", p=P))
    # sel8[dt]: (8,128) broadcast matrix, row 2dt+s has 1s in cols [64s,64s+64)
    sel8t = cpool.tile([8, 4, P], mybir.dt.float32r, name="sel8t")
    nc.sync.dma_start(sel8t[:], sel_d.rearrange("(d s) n -> s d n", s=8))
    sel8 = [sel8t[:, _dt, :] for _dt in range(4)]

    kvt = [kvp.tile([P, P], F32, tag="acc", name=f"kvt{_dt}")
           for _dt in range(DT)]
    zp = [None] * NCH
    zrA = misc.tile([8, L], mybir.dt.float32r, tag="zrA", name="zrA")

    def kv_mms(nt):
        vt, kft = vps[nt % 3], kf[nt]
        for dt in range(DT):
            nc.tensor.matmul(
                kvt[dt][:],
                kft[:, dt * P:(dt + 1) * P],
                vt[:, dt * P:(dt + 1) * P],
                start=(nt == 0), stop=(nt == NT - 1),
            )

    vps = [None] * 3
    for nt in range(NT):
        ps = pp.tile([P, 512], F32, tag="pp")
        for kc in range(KC):
            nc.tensor.matmul(
                ps[:],
                xva[:, kc, nt * P:(nt + 1) * P],
                wva[:, kc, :],
                start=(kc == 0), stop=(kc == KC - 1),
            )
        vp_t = vpp.tile([P, 512], BF16, tag="vp")
        nc.scalar.copy(vp_t[:], ps[:])
        vps[nt % 3] = vp_t
        if nt == 0:
            ksumS = ksum_tail()
        # KV matmuls staggered one n-tile behind the projections so the PE
        # never waits on the Vp copy
        if nt > 0:
            kv_mms(nt - 1)
        # interleaved Z chain (inputs qft/ksumS ready since phase Q/K)
        if 2 <= nt < 6:
            # Zpre group for n-chunk nt-2: accumulate 4 masked stationaries
            nch = nt - 2
            zp[nch] = pp.tile([8, 512], F32, tag="pp", name=f"zp{nch}")
            for dt in range(DT):
                nc.tensor.matmul(
                    zp[nch][:], ksumS[dt][:],
                    qft[dt * NCH + nch][:],
                    start=(dt == 0), stop=(dt == DT - 1),
                )
        elif 6 <= nt < 10:
            nch = nt - 6
            with nc.allow_low_precision(reason="zr in tf32 is plenty"):
                nc.vector.reciprocal(zrA[:, nch * 512:(nch + 1) * 512],
                                     zp[nch][:])
        elif nt >= 10:
            # zr broadcast (PE, f32r) + in-place QfT scale
            lo, hi = 3 * (nt - 10), min(16, 3 * (nt - 10) + 3)
            for idx in range(lo, hi):
                dt, nch = idx // NCH, idx % NCH
                zrp = pp.tile([P, 512], F32, tag="pp", name="zrp")
                nc.tensor.matmul(
                    zrp[:], sel8[dt][:],
                    zrA[:, nch * 512:(nch + 1) * 512],
                    start=True, stop=True,
                )
                qt = qft[dt * NCH + nch]
                nc.vector.tensor_tensor(qt[:], qt[:], zrp[:], Alu.mult)
    kv_mms(NT - 1)

    # kvcat[dt] = KV block-diagonal via mask (bf16 for the outT stationary)
    kvcat = []
    for dt in range(DT):
        kvc = misc.tile([P, P], BF16, tag="kvcat", bufs=4)
        nc.vector.tensor_tensor(kvc[:], kvt[dt][:], blkmask[:], Alu.mult)
        kvcat.append(kvc)

    # ---------------- transposed out ----------------
    # outT[(dt, nch)]: (128 m'-part, 512 n) = kvcat[dt]^T @ QfTz
    outT = [[None] * DT for _ in range(NCH)]
    for dt in range(DT):
        for nch in range(NCH):
            otp = pp.tile([P, 512], F32, tag="pp", name="otp")
            nc.tensor.matmul(
                otp[:], kvcat[dt][:], qft[dt * NCH + nch][:],
                start=True, stop=True,
            )
            oT = kfp.tile([P, 512], BF16, tag="kf", name="oT")
            if (nch + dt) % 2 == 0:
                nc.vector.tensor_copy(oT[:], otp[:])
            else:
                nc.scalar.copy(oT[:], otp[:])
            outT[nch][dt] = oT

    # ---------------- final projection ----------------
    # yT = WoS @ out_g^T: wo blocks stationary, reused across the 4 n-chunks
    for jb in range(8):
        ypool, ytag = ((kvp, "acc") if jb % 2 == 0 else (pp, "pp"))
        yps = [ypool.tile([P, 512], F32, tag=ytag, name=f"yp{_n}")
               for _n in range(NCH)]
        for dc in range(DT):
            for nch in range(NCH):
                nc.tensor.matmul(
                    yps[nch][:],
                    woa[:, dc, jb * P:(jb + 1) * P],
                    outT[nch][dc][:],
                    start=(dc == 0), stop=(dc == DT - 1),
                )
        yt = ysb.tile([P, L], BF16, tag="ysb", name="yt")
        for nch in range(NCH):
            if (jb + nch) % 2 == 0:
                nc.vector.tensor_copy(
                    yt[:, nch * 512:(nch + 1) * 512], yps[nch][:])
            else:
                nc.scalar.copy(
                    yt[:, nch * 512:(nch + 1) * 512], yps[nch][:])
            if jb == 7:
                # split the last row-block's writeback so it drains with
                # the copies instead of after them
                nc.sync.dma_start(
                    y_d[jb * P:(jb + 1) * P, nch * 512:(nch + 1) * 512],
                    yt[:, nch * 512:(nch + 1) * 512])
        if jb < 7:
            nc.sync.dma_start(y_d[jb * P:(jb + 1) * P, :], yt[:])


def make_in_maps(q, k, v, w_q, w_k, w_v, w_o):
    bf16 = mybir.dt.np(BF16)
    q = np.asarray(q, dtype=np.float32)
    k = np.asarray(k, dtype=np.float32)
    v = np.asarray(v, dtype=np.float32)
    w_q = np.asarray(w_q, dtype=np.float32)
    w_k = np.asarray(w_k, dtype=np.float32)
    w_v = np.asarray(w_v, dtype=np.float32)
    w_o = np.asarray(w_o, dtype=np.float32)
    B = q.shape[0]
    xqT = [np.ascontiguousarray(q[b].T).astype(bf16) for b in range(B)]
    xkT = [np.ascontiguousarray(k[b].T).astype(bf16) for b in range(B)]
    xvT = [np.ascontiguousarray(v[b].T).astype(bf16) for b in range(B)]
    wqT = [np.ascontiguousarray(w_q[g * DG:(g + 1) * DG, :].T).astype(bf16)
           for g in range(2)]
    wkT = [np.ascontiguousarray(w_k[g * DG:(g + 1) * DG, :].T).astype(bf16)
           for g in range(2)]
    wvT = [np.ascontiguousarray(w_v[g * DG:(g + 1) * DG, :].T).astype(bf16)
           for g in range(2)]
    woT = [np.ascontiguousarray(w_o[:, g * DG:(g + 1) * DG].T).astype(bf16)
           for g in range(2)]
    sel8 = np.zeros((32, P), dtype=np.float32)
    for dt in range(4):
        sel8[dt * 8 + 2 * dt, 0:64] = 1.0
        sel8[dt * 8 + 2 * dt + 1, 64:128] = 1.0
    in_maps = []
    for c in range(8):
        b, g = c // 2, c % 2
        in_maps.append({
            "xqT": xqT[b], "xkT": xkT[b], "xvT": xvT[b],
            "wqT": wqT[g], "wkT": wkT[g], "wvT": wvT[g], "woT": woT[g],
            "sel8": sel8,
        })
    return in_maps


def kernel(q, k, v, mask, w_q, w_k, w_v, w_o):
    if "nc" not in _CACHE:
        _CACHE["nc"] = build_nc()
    nc = _CACHE["nc"]
    in_maps = make_in_maps(q, k, v, w_q, w_k, w_v, w_o)
    res = run_bass_kernel_spmd(nc, in_maps, list(range(8)))
    _CACHE["last_results"] = res
    B = np.asarray(q).shape[0]
    out = np.empty((B, L, DM), dtype=np.float32)
    for b in range(B):
        out[b] = (res.results[2 * b]["y"].astype(np.float32)
                  + res.results[2 * b + 1]["y"].astype(np.float32)).T
    return out


# revision 35
# speedup vs baseline: 2.5584x; 1.3681x over previous
"""Linear-attention MultiHeadAttentionBlock kernel for 8 Trainium2 NeuronCores.

Sharding: core c handles (batch b = c//2, head-group g = c%2).  Each core
computes, for its batch's q/k/v and its 8 heads (512 of the 1024 d_model
dims):
    QfT = elu(Wq_g @ X_q^T)+1          (transposed space: d' on partitions)
    Kf  = elu(X_k @ Wk_g^T)+1          (n-space)
    ksum= ones^T-stationary accumulate over n-tiles -> (2, 512) -> 4 PE
          transposes -> ksumS[dt] (128, 8) masked stationaries
    Vp  = X_v @ Wv_g^T                 (n-space)
    KV  = per head-pair dt: Kf_dt^T @ Vp_dt (128x128, PE-accumulated over n)
    Zpre= ksumS[dt]-stationary matmuls vs QfT -> (8, L) head-major
    zr  = 1/Zpre (DVE), broadcast to (128, L) per dt on GpSimd,
          QfT *= zr (DVE, in place)
    kvcat = KV * block-diag mask (DVE)
    outT = kvcat_dt^T @ QfTz           (m'-space)
    y    = WoS @ out_g^T               (partial d_model-1024 output, bf16)
Host upcasts and sums the two per-batch partials.

All matmul operands are bf16 (halves HBM traffic vs fp32; PE row rate is
identical to fp32r).  PSUM accumulation stays fp32.
"""

import numpy as np

import concourse.bass as bass
import concourse.mybir as mybir
import concourse.tile as tile
from concourse import bacc
from concourse.bass_utils import run_bass_kernel_spmd
from concourse.masks import make_identity

P = 128
L = 2048          # sequence length
DM = 1024         # d_model (= contraction dim of projections)
DG = 512          # per-core head-group width (8 heads x 64)
NT = L // P       # 16 n-tiles
KC = DM // P      # 8 contraction chunks
DT = DG // P      # 4 d'-tiles (2 heads each)
NCH = 4           # n-chunks of 512
F32 = mybir.dt.float32
BF16 = mybir.dt.bfloat16

_CACHE = {}


def build_nc(repeats=1):
    nc = bacc.Bacc(None, target_bir_lowering=False)

    xq_d = nc.dram_tensor("xqT", [DM, L], BF16, kind="ExternalInput")
    xk_d = nc.dram_tensor("xkT", [DM, L], BF16, kind="ExternalInput")
    xv_d = nc.dram_tensor("xvT", [DM, L], BF16, kind="ExternalInput")
    wq_d = nc.dram_tensor("wqT", [DM, DG], BF16, kind="ExternalInput")
    wk_d = nc.dram_tensor("wkT", [DM, DG], BF16, kind="ExternalInput")
    wv_d = nc.dram_tensor("wvT", [DM, DG], BF16, kind="ExternalInput")
    wo_d = nc.dram_tensor("woT", [DG, DM], BF16, kind="ExternalInput")
    sel_d = nc.dram_tensor("sel8", [32, P], mybir.dt.float32r,
                           kind="ExternalInput")
    y_d = nc.dram_tensor("y", [DM, L], BF16, kind="ExternalOutput")

    with tile.TileContext(nc) as tc:
        with (
            tc.tile_pool(name="const", bufs=1) as cpool,
            tc.tile_pool(name="xt", bufs=2) as xt,       # (128,8,2048) x tensors
            tc.tile_pool(name="wt", bufs=2) as wt,       # (128,8,512) weights
            tc.tile_pool(name="wo", bufs=1) as wop,      # (128,4,1024) w_o
            tc.tile_pool(name="qft", bufs=16) as qftp,   # QfT persistent
            tc.tile_pool(name="kf", bufs=16) as kfp,     # Kf, later outT
            tc.tile_pool(name="vp", bufs=3) as vpp,      # Vp rotating
            tc.tile_pool(name="tmp", bufs=8) as tmp,     # feature-map temps
            tc.tile_pool(name="misc", bufs=1) as misc,   # zrA/ksumS/kvcat/ksA_sb
            tc.tile_pool(name="ysb", bufs=2) as ysb,     # (128,2048) y row-batches
            tc.tile_pool(name="pp", bufs=4, space="PSUM") as pp,
            tc.tile_pool(name="kvp", bufs=4, space="PSUM") as kvp,
        ):
            ident = cpool.tile([P, P], F32, name="ident")
            make_identity(nc, ident[:])
            ones_f = cpool.tile([P, 2], F32, name="ones_f")
            nc.gpsimd.memset(ones_f[:], 1.0)
            ones2 = cpool.tile([P, 2], BF16, name="ones2")
            nc.vector.tensor_copy(ones2[:], ones_f[:])
            # block-diag (128,128) mask: 1 where (i<64)==(j<64)
            bm_f = cpool.tile([P, P], F32, name="bm_f")
            nc.gpsimd.memset(bm_f[:], 0.0)
            nc.gpsimd.memset(bm_f[0:64, 0:64], 1.0)
            nc.gpsimd.memset(bm_f[64:128, 64:128], 1.0)
            blkmask = cpool.tile([P, P], BF16, name="blkmask")
            nc.vector.tensor_copy(blkmask[:], bm_f[:])
            for _rep in range(repeats):
                body(nc, tc, ident, ones2, blkmask, cpool,
                     xt, wt, wop, qftp, kfp, vpp, tmp, misc, ysb, pp, kvp,
                     xq_d, xk_d, xv_d, wq_d, wk_d, wv_d, wo_d, sel_d, y_d)

    nc.compile()
    return nc


def body(nc, tc, ident, ones2, blkmask, cpool,
         xt, wt, wop, qftp, kfp, vpp, tmp, misc, ysb, pp, kvp,
         xq_d, xk_d, xv_d, wq_d, wk_d, wv_d, wo_d, sel_d, y_d):
    Exp = mybir.ActivationFunctionType.Exp
    Relu = mybir.ActivationFunctionType.Relu
    Alu = mybir.AluOpType

    def feature_map(ps, dst):
        # dst = elu(ps)+1 = exp(min(ps,0)) + relu(ps)
        # (spread over DVE/ACT/ACT/Pool to keep per-engine queues short)
        t0 = tmp.tile([P, 512], F32, tag="tmp", name="t0")
        t1 = tmp.tile([P, 512], F32, tag="tmp", name="t1")
        nc.vector.tensor_scalar(t0[:], ps[:], 0.0, None, Alu.min)
        nc.scalar.activation(t1[:], ps[:], Relu)
        nc.scalar.activation(dst[:], t0[:], Exp)
        nc.vector.tensor_tensor(dst[:], dst[:], t1[:], Alu.add)

    def dma_x_all(src_d, name, npieces=1):
        # all 8 chunks as one (128, 8, L) tile; optionally split the DMA
        # into kc-groups so early chunks land before the full transfer
        t = xt.tile([P, KC, L], BF16, tag="xa", name=name, bufs=2)
        src = src_d.rearrange("(c p) n -> p c n", p=P)
        step = KC // npieces
        for i in range(0, KC, step):
            nc.sync.dma_start(t[:, i:i + step, :], src[:, i:i + step, :])
        return t

    def dma_w_all(src_d, name, npieces=1):
        # all 8 weight chunks as one (128, 8, DG) tile
        t = wt.tile([P, KC, DG], BF16, tag="wt", name=name, bufs=2)
        src = src_d.rearrange("(c p) n -> p c n", p=P)
        step = KC // npieces
        for i in range(0, KC, step):
            nc.sync.dma_start(t[:, i:i + step, :], src[:, i:i + step, :])
        return t

    # ---------------- Phase Q: QfT (transposed space) ----------------
    # split + interleave the first x/w transfers so chunk kc=0 lands quickly
    xqa = xt.tile([P, KC, L], BF16, tag="xa", name="xq", bufs=2)
    wqa = wt.tile([P, KC, DG], BF16, tag="wt", name="wq", bufs=2)
    xq_src = xq_d.rearrange("(c p) n -> p c n", p=P)
    wq_src = wq_d.rearrange("(c p) n -> p c n", p=P)
    nc.sync.dma_start(wqa[:, 0:1, :], wq_src[:, 0:1, :])
    nc.sync.dma_start(xqa[:, 0:1, 0:1024], xq_src[:, 0:1, 0:1024])
    nc.sync.dma_start(xqa[:, 0:1, 1024:2048], xq_src[:, 0:1, 1024:2048])
    nc.sync.dma_start(wqa[:, 1:3, :], wq_src[:, 1:3, :])
    nc.sync.dma_start(xqa[:, 1:2, :], xq_src[:, 1:2, :])
    nc.sync.dma_start(wqa[:, 3:8, :], wq_src[:, 3:8, :])
    nc.sync.dma_start(xqa[:, 2:4, :], xq_src[:, 2:4, :])
    nc.sync.dma_start(xqa[:, 4:6, :], xq_src[:, 4:6, :])
    nc.sync.dma_start(xqa[:, 6:8, :], xq_src[:, 6:8, :])

    qft = [None] * 16  # (128, 512) tiles: index dt*NCH + nch
    for dt in range(DT):
        ypool, ytag = ((pp, "pp") if dt % 2 == 0 else (kvp, "acc"))
        psq = [ypool.tile([P, 512], F32, tag=ytag, name=f"psq{_n}")
               for _n in range(NCH)]
        for kc in range(KC):
            for nch in range(NCH):
                nc.tensor.matmul(
                    psq[nch][:],
                    wqa[:, kc, dt * P:(dt + 1) * P],
                    xqa[:, kc, nch * 512:(nch + 1) * 512],
                    start=(kc == 0), stop=(kc == KC - 1),
                )
        for nch in range(NCH):
            qf = qftp.tile([P, 512], BF16, tag="qft")
            feature_map(psq[nch], qf)
            qft[dt * NCH + nch] = qf

    # ---------------- Phase K: Kf (n-space) + ksum ----------------
    xka = dma_x_all(xk_d, "xk")
    wka = dma_w_all(wk_d, "wk")
    kf = []
    ksA = kvp.tile([2, 512], F32, tag="acc", name="ksA")
    for nt in range(NT):
        ps = pp.tile([P, 512], F32, tag="pp")
        for kc in range(KC):
            nc.tensor.matmul(
                ps[:],
                xka[:, kc, nt * P:(nt + 1) * P],
                wka[:, kc, :],
                start=(kc == 0), stop=(kc == KC - 1),
            )
        kft = kfp.tile([P, 512], BF16, tag="kf")
        feature_map(ps, kft)
        kf.append(kft)
        # ksum accumulate: (2,512) += ones2^T @ Kf_(nt-1), staggered one
        # n-tile behind the projections so the PE never waits on the
        # feature-map chain
        if nt > 0:
            nc.tensor.matmul(
                ksA[:], ones2[:], kf[nt - 1][:],
                start=(nt == 1), stop=False,
            )

    def ksum_tail():
        # last ksA accumulate + ksum -> d'-partition masked stationaries
        # ksumS[dt] (128, 8) bf16.  Emitted early in phase V so the PE is
        # never parked on kf[15]'s feature-map chain.
        nc.tensor.matmul(ksA[:], ones2[:], kf[NT - 1][:],
                         start=False, stop=True)
        ksA_sb = misc.tile([2, 512], F32, tag="ksA_sb", name="ksA_sb")
        nc.scalar.copy(ksA_sb[:], ksA[0:2, :])
        ksumS = []
        for dt in range(DT):
            ztp = pp.tile([P, 2], F32, tag="pp", name="ztp")
            nc.tensor.transpose(ztp[:], ksA_sb[0:2, dt * P:(dt + 1) * P],
                                ident[0:2, 0:2])
            ks = misc.tile([P, 8], BF16, tag="ksumS", name=f"ksumS{dt}",
                           bufs=4)
            nc.gpsimd.memset(ks[:], 0.0)
            nc.scalar.copy(ks[0:64, 2 * dt:2 * dt + 1], ztp[0:64, 0:1])
            nc.scalar.copy(ks[64:128, 2 * dt + 1:2 * dt + 2],
                           ztp[64:128, 0:1])
            ksumS.append(ks)
        return ksumS

    # ---------------- Phase V: Vp + KV accumulation + Z chain ----------------
    xva = dma_x_all(xv_d, "xv")
    wva = dma_w_all(wv_d, "wv")
    woa = wop.tile([P, DT, DM], BF16, tag="wo", name="wo_t", bufs=1)
    nc.sync.dma_start(woa[:], wo_d.rearrange("(c p) n -> p c n", p=P))
    # sel8[dt]: (8,128) broadcast matrix, row 2dt+s has 1s in cols [64s,64s+64)
    sel8t = cpool.tile([8, 4, P], mybir.dt.float32r, name="sel8t")
    nc.sync.dma_start(sel8t[:], sel_d.rearrange("(d s) n -> s d n", s=8))
    sel8 = [sel8t[:, _dt, :] for _dt in range(4)]

    kvt = [kvp.tile([P, P], F32, tag="acc", name=f"kvt{_dt}")
           for _dt in range(DT)]
    zp = [None] * NCH
    zrA = misc.tile([8, L], mybir.dt.float32r, tag="zrA", name="zrA")

    def kv_mms(nt):
        vt, kft = vps[nt % 3], kf[nt]
        for dt in range(DT):
            nc.tensor.matmul(
                kvt[dt][:],
                kft[:, dt * P:(dt + 1) * P],
                vt[:, dt * P:(dt + 1) * P],
                start=(nt == 0), stop=(nt == NT - 1),
            )

    vps = [None] * 3
    for nt in range(NT):
        ps = pp.tile([P, 512], F32, tag="pp")
        for kc in range(KC):
            nc.tensor.matmul(
                ps[:],
                xva[:, kc, nt * P:(nt + 1) * P],
                wva[:, kc, :],
                start=(kc == 0), stop=(kc == KC - 1),
            )
        vp_t = vpp.tile([P, 512], BF16, tag="vp")
        nc.scalar.copy(vp_t[:], ps[:])
        vps[nt % 3] = vp_t
        if nt == 0:
            ksumS = ksum_tail()
        # KV matmuls staggered one n-tile behind the projections so the PE
        # never waits on the Vp copy
        if nt > 0:
            kv_mms(nt - 1)
        # interleaved Z chain (inputs qft/ksumS ready since phase Q/K)
        if 2 <= nt < 6:
            # Zpre group for n-chunk nt-2: accumulate 4 masked stationaries
            nch = nt - 2
            zp[nch] = pp.tile([8, 512], F32, tag="pp", name=f"zp{nch}")
            for dt in range(DT):
                nc.tensor.matmul(
                    zp[nch][:], ksumS[dt][:],
                    qft[dt * NCH + nch][:],
                    start=(dt == 0), stop=(dt == DT - 1),
                )
        elif 6 <= nt < 10:
            nch = nt - 6
            with nc.allow_low_precision(reason="zr in tf32 is plenty"):
                nc.vector.reciprocal(zrA[:, nch * 512:(nch + 1) * 512],
                                     zp[nch][:])
        elif nt >= 10:
            # zr broadcast (PE, f32r) + in-place QfT scale
            lo, hi = 3 * (nt - 10), min(16, 3 * (nt - 10) + 3)
            for idx in range(lo, hi):
                dt, nch = idx // NCH, idx % NCH
                zrp = pp.tile([P, 512], F32, tag="pp", name="zrp")
                nc.tensor.matmul(
                    zrp[:], sel8[dt][:],
                    zrA[:, nch * 512:(nch + 1) * 512],
                    start=True, stop=True,
                )
                qt = qft[dt * NCH + nch]
                nc.vector.tensor_tensor(qt[:], qt[:], zrp[:], Alu.mult)
    kv_mms(NT - 1)

    # kvcat[dt] = KV block-diagonal via mask (bf16 for the outT stationary)
    kvcat = []
    for dt in range(DT):
        kvc = misc.tile([P, P], BF16, tag="kvcat", bufs=4)
        nc.vector.tensor_tensor(kvc[:], kvt[dt][:], blkmask[:], Alu.mult)
        kvcat.append(kvc)

    # ---------------- transposed out ----------------
    # outT[(dt, nch)]: (128 m'-part, 512 n) = kvcat[dt]^T @ QfTz
    outT = [[None] * DT for _ in range(NCH)]
    for dt in range(DT):
        for nch in range(NCH):
            otp = pp.tile([P, 512], F32, tag="pp", name="otp")
            nc.tensor.matmul(
                otp[:], kvcat[dt][:], qft[dt * NCH + nch][:],
                start=True, stop=True,
            )
            oT = kfp.tile([P, 512], BF16, tag="kf", name="oT")
            if (nch + dt) % 2 == 0:
                nc.vector.tensor_copy(oT[:], otp[:])
            else:
                nc.scalar.copy(oT[:], otp[:])
            outT[nch][dt] = oT

    # ---------------- final projection ----------------
    # yT = WoS @ out_g^T: wo blocks stationary, reused across the 4 n-chunks
    for jb in range(8):
        ypool, ytag = ((kvp, "acc") if jb % 2 == 0 else (pp, "pp"))
        yps = [ypool.tile([P, 512], F32, tag=ytag, name=f"yp{_n}")
               for _n in range(NCH)]
        for dc in range(DT):
            for nch in range(NCH):
                nc.tensor.matmul(
                    yps[nch][:],
                    woa[:, dc, jb * P:(jb + 1) * P],
                    outT[nch][dc][:],
                    start=(dc == 0), stop=(dc == DT - 1),
                )
        yt = ysb.tile([P, L], BF16, tag="ysb", name="yt")
        for nch in range(NCH):
            if (jb + nch) % 2 == 0:
                nc.vector.tensor_copy(
                    yt[:, nch * 512:(nch + 1) * 512], yps[nch][:])
            else:
                nc.scalar.copy(
                    yt[:, nch * 512:(nch + 1) * 512], yps[nch][:])
        nc.sync.dma_start(y_d[jb * P:(jb + 1) * P, :], yt[:])


def make_in_maps(q, k, v, w_q, w_k, w_v, w_o):
    bf16 = mybir.dt.np(BF16)
    q = np.asarray(q, dtype=np.float32)
    k = np.asarray(k, dtype=np.float32)
    v = np.asarray(v, dtype=np.float32)
    w_q = np.asarray(w_q, dtype=np.float32)
    w_k = np.asarray(w_k, dtype=np.float32)
    w_v = np.asarray(w_v, dtype=np.float32)
    w_o = np.asarray(w_o, dtype=np.float32)
    B = q.shape[0]
    xqT = [np.ascontiguousarray(q[b].T).astype(bf16) for b in range(B)]
    xkT = [np.ascontiguousarray(k[b].T).astype(bf16) for b in range(B)]
    xvT = [np.ascontiguousarray(v[b].T).astype(bf16) for b in range(B)]
    wqT = [np.ascontiguousarray(w_q[g * DG:(g + 1) * DG, :].T).astype(bf16)
           for g in range(2)]
    wkT = [np.ascontiguousarray(w_k[g * DG:(g + 1) * DG, :].T).astype(bf16)
           for g in range(2)]
    wvT = [np.ascontiguousarray(w_v[g * DG:(g + 1) * DG, :].T).astype(bf16)
           for g in range(2)]
    woT = [np.ascontiguousarray(w_o[:, g * DG:(g + 1) * DG].T).astype(bf16)
           for g in range(2)]
    sel8 = np.zeros((32, P), dtype=np.float32)
    for dt in range(4):
        sel8[dt * 8 + 2 * dt, 0:64] = 1.0
        sel8[dt * 8 + 2 * dt + 1, 64:128] = 1.0
    in_maps = []
    for c in range(8):
        b, g = c // 2, c % 2
        in_maps.append({
            "xqT": xqT[b], "xkT": xkT[b], "xvT": xvT[b],
            "wqT": wqT[g], "wkT": wkT[g], "wvT": wvT[g], "woT": woT[g],
            "sel8": sel8,
        })
    return in_maps


def kernel(q, k, v, mask, w_q, w_k, w_v, w_o):
    if "nc" not in _CACHE:
        _CACHE["nc"] = build_nc()
    nc = _CACHE["nc"]
    in_maps = make_in_maps(q, k, v, w_q, w_k, w_v, w_o)
    res = run_bass_kernel_spmd(nc, in_maps, list(range(8)))
    _CACHE["last_results"] = res
    B = np.asarray(q).shape[0]
    out = np.empty((B, L, DM), dtype=np.float32)
    for b in range(B):
        out[b] = (res.results[2 * b]["y"].astype(np.float32)
                  + res.results[2 * b + 1]["y"].astype(np.float32)).T
    return out
